# revision 4
# baseline (speedup 1.0000x reference)
"""Multi-head attention (B=8, C=512, L=2048, H=8, D=64) on 8 TRN2 NeuronCores.

Sharding: pure batch-parallel - core b computes batch b end-to-end (qkv proj,
8 heads of attention, out proj). No collectives.

Per-core layout strategy:
  - qkv projection with lhsT = w_qkv.T (host-transposed), rhs = x.
  - S^T = K^T Q  (keys on partitions) so softmax exp output is already the
    transposed P^T needed by the PV matmul, and no max-subtraction is needed
    (scores are ~N(0,1) after the 1/sqrt(D) scale, folded into exp's scale).
  - PV uses lhsT = [V^T | ones] (65 columns): row 64 of the accumulator is
    the softmax denominator, computed for free.
  - V^T is computed directly from X (lhsT = X tiles), V is never materialized.
  - S^T matmuls are row-packed in K=64 pairs (array rows 0-63 / 64-127) using
    partition-duplicated Q/K tiles, for full PE utilization.
"""

import os
import sys

sys.path.insert(0, "/opt/trn_rl_repo")

import numpy as np
import ml_dtypes

import concourse.bass as bass
import concourse.tile as tile
from concourse import bacc, mybir
from concourse import bass_utils

B, C, L = 8, 512, 2048
H, D = 8, 64
HID = H * D  # 512
SCALE = float(D) ** -0.5
BF16 = mybir.dt.bfloat16
F32 = mybir.dt.float32
AF = mybir.ActivationFunctionType
NCORES = 8

NT = C // 128  # 4 channel tiles
NL = L // 512  # 4 l-chunks of 512
NJ = L // 128  # 16 key tiles


def build_kernel(tc, out_d, x_d, wqkvT_d, woutT_d, bias_d):
    nc = tc.nc
    from contextlib import ExitStack

    ctx = ExitStack()
    pers = ctx.enter_context(tc.tile_pool(name="pers", bufs=1))
    stg = ctx.enter_context(tc.tile_pool(name="stg", bufs=2))
    ptp = ctx.enter_context(tc.tile_pool(name="ptp", bufs=6))
    ytp = ctx.enter_context(tc.tile_pool(name="ytp", bufs=3))
    smp = ctx.enter_context(tc.tile_pool(name="smp", bufs=2))
    stp = ctx.enter_context(tc.tile_pool(name="stp", bufs=2, space="PSUM"))
    otp = ctx.enter_context(tc.tile_pool(name="otp", bufs=1, space="PSUM"))
    qkp = ctx.enter_context(tc.tile_pool(name="qkp", bufs=2, space="PSUM"))

    # ---- persistent SBUF tensors ----
    x_sb = [pers.tile([128, L], BF16, tag=f"x{c}", name=f"x{c}") for c in range(NT)]
    wq_sb = [
        pers.tile([128, 3 * HID], BF16, tag=f"wq{c}", name=f"wq{c}") for c in range(NT)
    ]
    wo_sb = [pers.tile([128, C], BF16, tag=f"wo{c}", name=f"wo{c}") for c in range(NT)]
    bias_sb = [
        pers.tile([128, 1], F32, tag=f"bias{c}", name=f"bias{c}") for c in range(NT)
    ]
    q2 = [pers.tile([128, L], BF16, tag=f"q2_{h}", name=f"q2_{h}") for h in range(H)]
    k2 = [pers.tile([128, L], BF16, tag=f"k2_{h}", name=f"k2_{h}") for h in range(H)]
    vt1 = [
        pers.tile([128, H * 65], BF16, tag=f"vt{j}", name=f"vt{j}") for j in range(NJ)
    ]
    o2 = [pers.tile([128, L], BF16, tag=f"o2_{c}", name=f"o2_{c}") for c in range(NT)]

    # ---- input DMAs ----
    for c in range(NT):
        nc.sync.dma_start(x_sb[c][:, :], x_d[128 * c : 128 * (c + 1), :])
        nc.sync.dma_start(wq_sb[c][:, :], wqkvT_d[128 * c : 128 * (c + 1), :])
        nc.sync.dma_start(wo_sb[c][:, :], woutT_d[128 * c : 128 * (c + 1), :])
        nc.sync.dma_start(bias_sb[c][:, :], bias_d[128 * c : 128 * (c + 1), :])

    def emit_qk_pair(t):
        """Project q and k rows 128t..128t+128 (heads 2t, 2t+1) and write the
        partition-duplicated q2/k2 tiles via SBUF->SBUF DMA."""
        for kind in range(2):  # 0 = q, 1 = k
            stage = stg.tile([128, L], BF16, tag="stage", name=f"stage_{kind}_{t}")
            ocol = kind * HID + 128 * t
            for n in range(NL):
                ps = qkp.tile([128, 512], F32, tag="qkp", name=f"qk_ps_{kind}_{t}_{n}")
                for c in range(NT):
                    nc.tensor.matmul(
                        ps[:, :],
                        lhsT=wq_sb[c][:, ocol : ocol + 128],
                        rhs=x_sb[c][:, 512 * n : 512 * (n + 1)],
                        start=(c == 0),
                        stop=(c == NT - 1),
                    )
                nc.vector.tensor_copy(stage[:, 512 * n : 512 * (n + 1)], ps[:, :])
            dsts = (q2, k2)[kind]
            d0, d1 = dsts[2 * t], dsts[2 * t + 1]
            nc.sync.dma_start(d0[0:64, :], stage[0:64, :])
            nc.sync.dma_start(d0[64:128, :], stage[0:64, :])
            nc.sync.dma_start(d1[0:64, :], stage[64:128, :])
            nc.sync.dma_start(d1[64:128, :], stage[64:128, :])

    def emit_vt(jt):
        """V^T tile for key-block jt: [128 keys, 8 heads x (64 dims + ones)]."""
        ps = qkp.tile([128, 512], F32, tag="qkp", name=f"vt_ps_{jt}")
        for c in range(NT):
            nc.tensor.matmul(
                ps[:, :],
                lhsT=x_sb[c][:, 128 * jt : 128 * (jt + 1)],
                rhs=wq_sb[c][:, 2 * HID : 3 * HID],
                start=(c == 0),
                stop=(c == NT - 1),
            )
        vv = vt1[jt].rearrange("p (h e) -> p h e", e=65)
        nc.vector.tensor_copy(vv[:, :, 0:64], ps.rearrange("p (h d) -> p h d", d=64))
        nc.vector.memset(vv[:, :, 64:65], 1.0)

    def emit_head(h, interleave):
        """Attention for head h. `interleave` is a list of closures emitting
        independent PE work (qk-proj of later pairs / vt tiles) to fill PE
        slack inside the exp-bound j-loop."""
        t = h // 2
        q2h, k2h = q2[h], k2[h]
        slot = 0
        for ih in range(2):
            ib = 1024 * ih
            ot = otp.tile([65, 1024], F32, tag="ot", name=f"ot_{h}_{ih}")
            for jt in range(NJ):
                st = stp.tile([128, 1024], F32, tag="st", name=f"st_{h}_{ih}_{jt}")
                # packed S^T pair: rows 0-63 compute i-chunk 0, rows 64-127
                # compute i-chunk 1 (concurrent in the PE array)
                nc.tensor.matmul(
                    st[:, 0:512],
                    lhsT=k2h[0:64, 128 * jt : 128 * (jt + 1)],
                    rhs=q2h[0:64, ib : ib + 512],
                    start=True,
                    stop=True,
                )
                nc.tensor.matmul(
                    st[:, 512:1024],
                    lhsT=k2h[64:128, 128 * jt : 128 * (jt + 1)],
                    rhs=q2h[64:128, ib + 512 : ib + 1024],
                    start=True,
                    stop=True,
                )
                pt = ptp.tile([128, 1024], BF16, tag="pt", name=f"pt_{h}_{ih}_{jt}")
                nc.scalar.activation(pt[:, :], st[:, :], AF.Exp, scale=SCALE)
                vt = vt1[jt]
                nc.tensor.matmul(
                    ot[:, 0:512],
                    lhsT=vt[:, 65 * h : 65 * h + 65],
                    rhs=pt[:, 0:512],
                    start=(jt == 0),
                    stop=(jt == NJ - 1),
                )
                nc.tensor.matmul(
                    ot[:, 512:1024],
                    lhsT=vt[:, 65 * h : 65 * h + 65],
                    rhs=pt[:, 512:1024],
                    start=(jt == 0),
                    stop=(jt == NJ - 1),
                )
                # fill PE slack with independent work
                if interleave and (jt % 2 == 1) and slot < len(interleave):
                    interleave[slot]()
                    slot += 1
            # softmax normalization: divide rows 0-63 by the ones-row (64).
            # NOTE: reciprocal_approx_fast mis-reads PSUM at partition offset
            # 64 on silicon (reads partition 0) - stage the row through SBUF.
            den = smp.tile([1, 1024], F32, tag="den", name=f"den_{h}_{ih}")
            nc.vector.tensor_copy(den[:, :], ot[64:65, :])
            rec = smp.tile([1, 1024], F32, tag="rec", name=f"rec_{h}_{ih}")
            nc.vector.reciprocal_approx_fast(rec[:, :], den[:, :])
            rb = smp.tile([64, 1024], F32, tag="rb", name=f"rb_{h}_{ih}")
            nc.gpsimd.partition_broadcast(rb[:, :], rec[:, :])
            dst = o2[t][(h % 2) * 64 : (h % 2) * 64 + 64, ib : ib + 1024]
            nc.vector.tensor_mul(dst, ot[0:64, :], rb[:, :])
        del interleave[: slot]

    def emit_proj():
        for o in range(NT):
            for n in range(NL):
                ps = qkp.tile([128, 512], F32, tag="qkp", name=f"y_ps_{o}_{n}")
                for c in range(NT):
                    nc.tensor.matmul(
                        ps[:, :],
                        lhsT=wo_sb[c][:, 128 * o : 128 * (o + 1)],
                        rhs=o2[c][:, 512 * n : 512 * (n + 1)],
                        start=(c == 0),
                        stop=(c == NT - 1),
                    )
                yt = ytp.tile([128, 512], F32, tag="yt", name=f"yt_{o}_{n}")
                nc.vector.tensor_scalar_add(yt[:, :], ps[:, :], bias_sb[o][:, 0:1])
                nc.sync.dma_start(
                    out_d[128 * o : 128 * (o + 1), 512 * n : 512 * (n + 1)], yt[:, :]
                )

    # ---- emission schedule ----
    emit_qk_pair(0)  # heads 0,1 projected up front
    for jt in range(NJ):
        emit_vt(jt)  # needed from head 0's first PV matmul

    # later qk pairs are interleaved into earlier heads' j-loops so the PE
    # fills its slack while ScalarE exp is the bottleneck
    iq = [lambda t=t: emit_qk_pair(t) for t in (1, 2, 3)]
    for h in range(H):
        inter = [iq.pop(0)] if (h in (1, 3, 5) and iq) else []
        emit_head(h, inter)
    emit_proj()
    ctx.close()


_COMPILED = None


def _get_compiled():
    global _COMPILED
    if _COMPILED is None:
        nc = bacc.Bacc(
            "TRN2", target_bir_lowering=False, debug=False, num_devices=NCORES
        )
        x_d = nc.dram_tensor("x", [C, L], BF16, kind="ExternalInput").ap()
        wqkvT_d = nc.dram_tensor("wqkvT", [C, 3 * HID], BF16, kind="ExternalInput").ap()
        woutT_d = nc.dram_tensor("woutT", [HID, C], BF16, kind="ExternalInput").ap()
        bias_d = nc.dram_tensor("bias", [C, 1], F32, kind="ExternalInput").ap()
        out_d = nc.dram_tensor("out", [C, L], F32, kind="ExternalOutput").ap()
        with tile.TileContext(nc) as tc:
            build_kernel(tc, out_d, x_d, wqkvT_d, woutT_d, bias_d)
        nc.compile()
        _COMPILED = nc
    return _COMPILED


def make_in_maps(x, w_qkv, w_out, b_out):
    xb = np.asarray(x, dtype=np.float32).astype(ml_dtypes.bfloat16)
    wqkvT = np.ascontiguousarray(
        np.asarray(w_qkv, dtype=np.float32).T.astype(ml_dtypes.bfloat16)
    )
    woutT = np.ascontiguousarray(
        np.asarray(w_out, dtype=np.float32).T.astype(ml_dtypes.bfloat16)
    )
    bias = np.ascontiguousarray(
        np.asarray(b_out, dtype=np.float32).reshape(C, 1)
    )
    return [
        {
            "x": np.ascontiguousarray(xb[b]),
            "wqkvT": wqkvT,
            "woutT": woutT,
            "bias": bias,
        }
        for b in range(B)
    ]


LAST_RESULTS = None


def _install_ntff_hook():
    """Provide antenv.axon_hooks (absent from this image) so trace=True works."""
    import types

    try:
        from antenv.axon_hooks import get_axon_ntff_profile_hook  # noqa: F401

        return
    except ImportError:
        pass
    sys.path.insert(0, "/root/.axon_site")
    from trn_agent_boot.trn_boot import _ntff_profile_via_ctypes

    hook = _ntff_profile_via_ctypes("/opt/axon/libaxon_pjrt.so")
    import antenv

    mod = types.ModuleType("antenv.axon_hooks")
    mod._hook = hook
    mod.get_axon_ntff_profile_hook = lambda: mod._hook
    mod.set_axon_ntff_profile_hook = lambda h: setattr(mod, "_hook", h)
    sys.modules["antenv.axon_hooks"] = mod
    antenv.axon_hooks = mod
    # artifact upload has no egress in this container - make it a no-op
    bass_utils.upload_artifacts = lambda tmpdir: tmpdir


def kernel(x, w_qkv, w_out, b_out):
    global LAST_RESULTS
    nc = _get_compiled()
    in_maps = make_in_maps(x, w_qkv, w_out, b_out)
    trace = bool(int(os.environ.get("KERNEL_TRACE", "0")))
    if trace:
        _install_ntff_hook()
    res = bass_utils.run_bass_kernel_spmd(
        nc, in_maps, core_ids=list(range(NCORES)), trace=trace
    )
    LAST_RESULTS = res
    out = np.stack([np.asarray(res.results[b]["out"]) for b in range(B)])
    return out.astype(np.float32)


# revision 8
# speedup vs baseline: 1.1682x; 1.1682x over previous
"""Multi-head attention (B=8, C=512, L=2048, H=8, D=64) on 8 TRN2 NeuronCores.

Sharding: pure batch-parallel - core b computes batch b end-to-end (qkv proj,
8 heads of attention, out proj). No collectives.

Per-core layout strategy:
  - qkv projection with lhsT = w_qkv.T (host-transposed), rhs = x.
  - S^T = K^T Q  (keys on partitions) so softmax exp output is already the
    transposed P^T needed by the PV matmul, and no max-subtraction is needed
    (scores are ~N(0,1) after the 1/sqrt(D) scale, folded into exp's scale).
  - PV uses lhsT = [V^T | ones] (65 columns): row 64 of the accumulator is
    the softmax denominator, computed for free.
  - V^T is computed directly from X (lhsT = X tiles), V is never materialized.
  - S^T matmuls are row-packed in K=64 pairs (array rows 0-63 / 64-127) using
    partition-duplicated Q/K tiles, for full PE utilization.
"""

import os
import sys

sys.path.insert(0, "/opt/trn_rl_repo")

import numpy as np
import ml_dtypes

import concourse.bass as bass
import concourse.tile as tile
from concourse import bacc, mybir
from concourse import bass_utils

B, C, L = 8, 512, 2048
H, D = 8, 64
HID = H * D  # 512
SCALE = float(D) ** -0.5
BF16 = mybir.dt.bfloat16
F32 = mybir.dt.float32
AF = mybir.ActivationFunctionType
NCORES = 8

NT = C // 128  # 4 channel tiles
NL = L // 512  # 4 l-chunks of 512
NJ = L // 128  # 16 key tiles


def build_kernel(tc, out_d, x_d, wqkvT_d, woutT_d, bias_d):
    nc = tc.nc
    from contextlib import ExitStack

    ctx = ExitStack()
    pers = ctx.enter_context(tc.tile_pool(name="pers", bufs=1))
    stg = ctx.enter_context(tc.tile_pool(name="stg", bufs=2))
    ptp = ctx.enter_context(tc.tile_pool(name="ptp", bufs=8))
    ytp = ctx.enter_context(tc.tile_pool(name="ytp", bufs=3))
    smp = ctx.enter_context(tc.tile_pool(name="smp", bufs=2))
    stp = ctx.enter_context(tc.tile_pool(name="stp", bufs=2, space="PSUM"))
    otp = ctx.enter_context(tc.tile_pool(name="otp", bufs=1, space="PSUM"))
    qkp = ctx.enter_context(tc.tile_pool(name="qkp", bufs=2, space="PSUM"))

    # ---- persistent SBUF tensors ----
    x_sb = [pers.tile([128, L], BF16, tag=f"x{c}", name=f"x{c}") for c in range(NT)]
    wq_sb = [
        pers.tile([128, 3 * HID], BF16, tag=f"wq{c}", name=f"wq{c}") for c in range(NT)
    ]
    wo_sb = [pers.tile([128, C], BF16, tag=f"wo{c}", name=f"wo{c}") for c in range(NT)]
    bias_sb = [
        pers.tile([128, 1], F32, tag=f"bias{c}", name=f"bias{c}") for c in range(NT)
    ]
    q2 = [pers.tile([128, L], BF16, tag=f"q2_{h}", name=f"q2_{h}") for h in range(H)]
    k2 = [pers.tile([128, L], BF16, tag=f"k2_{h}", name=f"k2_{h}") for h in range(H)]
    vt1 = [
        pers.tile([128, H * 65], BF16, tag=f"vt{j}", name=f"vt{j}") for j in range(NJ)
    ]
    o2 = [pers.tile([128, L], BF16, tag=f"o2_{c}", name=f"o2_{c}") for c in range(NT)]

    # ---- input DMAs ----
    for c in range(NT):
        nc.sync.dma_start(x_sb[c][:, :], x_d[128 * c : 128 * (c + 1), :])
        nc.sync.dma_start(wq_sb[c][:, :], wqkvT_d[128 * c : 128 * (c + 1), :])
        nc.sync.dma_start(wo_sb[c][:, :], woutT_d[128 * c : 128 * (c + 1), :])
        nc.sync.dma_start(bias_sb[c][:, :], bias_d[128 * c : 128 * (c + 1), :])

    def emit_qk_pair(t):
        """Project q and k rows 128t..128t+128 (heads 2t, 2t+1) and write the
        partition-duplicated q2/k2 tiles via SBUF->SBUF DMA."""
        for kind in range(2):  # 0 = q, 1 = k
            stage = stg.tile([128, L], BF16, tag="stage", name=f"stage_{kind}_{t}")
            ocol = kind * HID + 128 * t
            for n in range(NL):
                ps = qkp.tile([128, 512], F32, tag="qkp", name=f"qk_ps_{kind}_{t}_{n}")
                for c in range(NT):
                    nc.tensor.matmul(
                        ps[:, :],
                        lhsT=wq_sb[c][:, ocol : ocol + 128],
                        rhs=x_sb[c][:, 512 * n : 512 * (n + 1)],
                        start=(c == 0),
                        stop=(c == NT - 1),
                    )
                nc.vector.tensor_copy(stage[:, 512 * n : 512 * (n + 1)], ps[:, :])
            dsts = (q2, k2)[kind]
            d0, d1 = dsts[2 * t], dsts[2 * t + 1]
            nc.sync.dma_start(d0[0:64, :], stage[0:64, :])
            nc.sync.dma_start(d0[64:128, :], stage[0:64, :])
            nc.sync.dma_start(d1[0:64, :], stage[64:128, :])
            nc.sync.dma_start(d1[64:128, :], stage[64:128, :])

    def emit_vt(jt):
        """V^T tile for key-block jt: [128 keys, 8 heads x (64 dims + ones)]."""
        ps = qkp.tile([128, 512], F32, tag="qkp", name=f"vt_ps_{jt}")
        for c in range(NT):
            nc.tensor.matmul(
                ps[:, :],
                lhsT=x_sb[c][:, 128 * jt : 128 * (jt + 1)],
                rhs=wq_sb[c][:, 2 * HID : 3 * HID],
                start=(c == 0),
                stop=(c == NT - 1),
            )
        vv = vt1[jt].rearrange("p (h e) -> p h e", e=65)
        nc.vector.tensor_copy(vv[:, :, 0:64], ps.rearrange("p (h d) -> p h d", d=64))
        nc.vector.memset(vv[:, :, 64:65], 1.0)

    def emit_head(h, ih, interleave):
        """Attention for head h, i-half ih. `interleave` is a list of closures
        emitting independent PE work (qk-proj of later pairs / proj groups) to
        fill PE slack inside the exp-bound j-loop."""
        t = h // 2
        q2h, k2h = q2[h], k2[h]
        slot = 0
        ib = 1024 * ih
        ot = otp.tile([65, 1024], F32, tag="ot", name=f"ot_{h}_{ih}")
        for jt in range(NJ):
            st = stp.tile([128, 1024], F32, tag="st", name=f"st_{h}_{ih}_{jt}")
            # packed S^T pair: rows 0-63 compute i-chunk 0, rows 64-127
            # compute i-chunk 1 (concurrent in the PE array)
            nc.tensor.matmul(
                st[:, 0:512],
                lhsT=k2h[0:64, 128 * jt : 128 * (jt + 1)],
                rhs=q2h[0:64, ib : ib + 512],
                start=True,
                stop=True,
            )
            nc.tensor.matmul(
                st[:, 512:1024],
                lhsT=k2h[64:128, 128 * jt : 128 * (jt + 1)],
                rhs=q2h[64:128, ib + 512 : ib + 1024],
                start=True,
                stop=True,
            )
            pt = ptp.tile([128, 1024], BF16, tag="pt", name=f"pt_{h}_{ih}_{jt}")
            nc.scalar.activation(pt[:, :], st[:, :], AF.Exp, scale=SCALE)
            vt = vt1[jt]
            nc.tensor.matmul(
                ot[:, 0:512],
                lhsT=vt[:, 65 * h : 65 * h + 65],
                rhs=pt[:, 0:512],
                start=(jt == 0),
                stop=(jt == NJ - 1),
            )
            nc.tensor.matmul(
                ot[:, 512:1024],
                lhsT=vt[:, 65 * h : 65 * h + 65],
                rhs=pt[:, 512:1024],
                start=(jt == 0),
                stop=(jt == NJ - 1),
            )
            # fill PE slack with independent work
            if interleave and (jt % 2 == 1) and slot < len(interleave):
                interleave[slot]()
                slot += 1
        # Evacuate the accumulator in ONE copy so the psum tile frees fast
        # (otherwise the next head's first PV stalls ~5us and the PE goes
        # HAM-cold); normalize afterwards from SBUF.
        # NOTE: reciprocal_approx_fast mis-reads PSUM at partition offset 64
        # on silicon (reads partition 0), so SBUF staging is also required.
        o2u = smp.tile([65, 1024], F32, tag="o2u", name=f"o2u_{h}_{ih}")
        nc.vector.tensor_copy(o2u[:, :], ot[:, :])
        den = smp.tile([1, 1024], F32, tag="den", name=f"den_{h}_{ih}")
        nc.vector.tensor_copy(den[:, :], o2u[64:65, :])
        rec = smp.tile([1, 1024], F32, tag="rec", name=f"rec_{h}_{ih}")
        nc.vector.reciprocal_approx_fast(rec[:, :], den[:, :])
        rb = smp.tile([64, 1024], F32, tag="rb", name=f"rb_{h}_{ih}")
        nc.gpsimd.partition_broadcast(rb[:, :], rec[:, :])
        dst = o2[t][(h % 2) * 64 : (h % 2) * 64 + 64, ib : ib + 1024]
        nc.vector.tensor_mul(dst, o2u[0:64, :], rb[:, :])
        del interleave[:slot]

    def emit_proj_group(o, n):
        ps = qkp.tile([128, 512], F32, tag="qkp", name=f"y_ps_{o}_{n}")
        for c in range(NT):
            nc.tensor.matmul(
                ps[:, :],
                lhsT=wo_sb[c][:, 128 * o : 128 * (o + 1)],
                rhs=o2[c][:, 512 * n : 512 * (n + 1)],
                start=(c == 0),
                stop=(c == NT - 1),
            )
        yt = ytp.tile([128, 512], F32, tag="yt", name=f"yt_{o}_{n}")
        nc.vector.tensor_scalar_add(yt[:, :], ps[:, :], bias_sb[o][:, 0:1])
        nc.sync.dma_start(
            out_d[128 * o : 128 * (o + 1), 512 * n : 512 * (n + 1)], yt[:, :]
        )

    # ---- emission schedule ----
    emit_qk_pair(0)  # heads 0,1 projected up front
    for jt in range(NJ):
        emit_vt(jt)  # needed from head 0's first PV matmul

    # ih-outer: pass 0 computes o2[:, 0:1024] for all heads, so the first
    # half of the output projection overlaps pass 1. qk pairs for later
    # heads are interleaved into earlier heads' j-loops so the PE fills its
    # slack while ScalarE exp is the bottleneck.
    iq = [lambda t=t: emit_qk_pair(t) for t in (1, 2, 3)]
    for h in range(H):
        inter = [iq.pop(0)] if (h in (1, 2, 3) and iq) else []
        emit_head(h, 0, inter)
    proj_half0 = [lambda o=o, n=n: emit_proj_group(o, n) for o in range(NT) for n in (0, 1)]
    for h in range(H):
        inter = [proj_half0.pop(0)] if proj_half0 else []
        emit_head(h, 1, inter)
    while proj_half0:
        proj_half0.pop(0)()
    for o in range(NT):
        for n in (2, 3):
            emit_proj_group(o, n)
    ctx.close()


_COMPILED = None


def _get_compiled():
    global _COMPILED
    if _COMPILED is None:
        nc = bacc.Bacc(
            "TRN2", target_bir_lowering=False, debug=False, num_devices=NCORES
        )
        x_d = nc.dram_tensor("x", [C, L], BF16, kind="ExternalInput").ap()
        wqkvT_d = nc.dram_tensor("wqkvT", [C, 3 * HID], BF16, kind="ExternalInput").ap()
        woutT_d = nc.dram_tensor("woutT", [HID, C], BF16, kind="ExternalInput").ap()
        bias_d = nc.dram_tensor("bias", [C, 1], F32, kind="ExternalInput").ap()
        out_d = nc.dram_tensor("out", [C, L], F32, kind="ExternalOutput").ap()
        with tile.TileContext(nc) as tc:
            build_kernel(tc, out_d, x_d, wqkvT_d, woutT_d, bias_d)
        nc.compile()
        _COMPILED = nc
    return _COMPILED


def make_in_maps(x, w_qkv, w_out, b_out):
    xb = np.asarray(x, dtype=np.float32).astype(ml_dtypes.bfloat16)
    wqkvT = np.ascontiguousarray(
        np.asarray(w_qkv, dtype=np.float32).T.astype(ml_dtypes.bfloat16)
    )
    woutT = np.ascontiguousarray(
        np.asarray(w_out, dtype=np.float32).T.astype(ml_dtypes.bfloat16)
    )
    bias = np.ascontiguousarray(
        np.asarray(b_out, dtype=np.float32).reshape(C, 1)
    )
    return [
        {
            "x": np.ascontiguousarray(xb[b]),
            "wqkvT": wqkvT,
            "woutT": woutT,
            "bias": bias,
        }
        for b in range(B)
    ]


LAST_RESULTS = None


def _install_ntff_hook():
    """Provide antenv.axon_hooks (absent from this image) so trace=True works."""
    import types

    try:
        from antenv.axon_hooks import get_axon_ntff_profile_hook  # noqa: F401

        return
    except ImportError:
        pass
    sys.path.insert(0, "/root/.axon_site")
    from trn_agent_boot.trn_boot import _ntff_profile_via_ctypes

    hook = _ntff_profile_via_ctypes("/opt/axon/libaxon_pjrt.so")
    import antenv

    mod = types.ModuleType("antenv.axon_hooks")
    mod._hook = hook
    mod.get_axon_ntff_profile_hook = lambda: mod._hook
    mod.set_axon_ntff_profile_hook = lambda h: setattr(mod, "_hook", h)
    sys.modules["antenv.axon_hooks"] = mod
    antenv.axon_hooks = mod
    # artifact upload has no egress in this container - make it a no-op
    bass_utils.upload_artifacts = lambda tmpdir: tmpdir


def kernel(x, w_qkv, w_out, b_out):
    global LAST_RESULTS
    nc = _get_compiled()
    in_maps = make_in_maps(x, w_qkv, w_out, b_out)
    trace = bool(int(os.environ.get("KERNEL_TRACE", "0")))
    if trace:
        _install_ntff_hook()
    res = bass_utils.run_bass_kernel_spmd(
        nc, in_maps, core_ids=list(range(NCORES)), trace=trace
    )
    LAST_RESULTS = res
    out = np.stack([np.asarray(res.results[b]["out"]) for b in range(B)])
    return out.astype(np.float32)


# revision 11
# speedup vs baseline: 1.1706x; 1.0020x over previous
"""Multi-head attention (B=8, C=512, L=2048, H=8, D=64) on 8 TRN2 NeuronCores.

Sharding: pure batch-parallel - core b computes batch b end-to-end (qkv proj,
8 heads of attention, out proj). No collectives.

Per-core layout strategy:
  - qkv projection with lhsT = w_qkv.T (host-transposed), rhs = x.
  - S^T = K^T Q  (keys on partitions) so softmax exp output is already the
    transposed P^T needed by the PV matmul, and no max-subtraction is needed
    (scores are ~N(0,1) after the 1/sqrt(D) scale, folded into exp's scale).
  - PV uses lhsT = [V^T | ones] (65 columns): row 64 of the accumulator is
    the softmax denominator, computed for free.
  - V^T is computed directly from X (lhsT = X tiles), V is never materialized.
  - S^T matmuls are row-packed in K=64 pairs (array rows 0-63 / 64-127) using
    partition-duplicated Q/K tiles, for full PE utilization.
"""

import os
import sys

sys.path.insert(0, "/opt/trn_rl_repo")

import numpy as np
import ml_dtypes

import concourse.bass as bass
import concourse.tile as tile
from concourse import bacc, mybir
from concourse import bass_utils

B, C, L = 8, 512, 2048
H, D = 8, 64
HID = H * D  # 512
SCALE = float(D) ** -0.5
BF16 = mybir.dt.bfloat16
F32 = mybir.dt.float32
AF = mybir.ActivationFunctionType
NCORES = 8

NT = C // 128  # 4 channel tiles
NL = L // 512  # 4 l-chunks of 512
NJ = L // 128  # 16 key tiles


def build_kernel(tc, out_d, x_d, wqkvT_d, woutT_d, bias_d):
    nc = tc.nc
    from contextlib import ExitStack

    ctx = ExitStack()
    pers = ctx.enter_context(tc.tile_pool(name="pers", bufs=1))
    stg = ctx.enter_context(tc.tile_pool(name="stg", bufs=2))
    ptp = ctx.enter_context(tc.tile_pool(name="ptp", bufs=8))
    ytp = ctx.enter_context(tc.tile_pool(name="ytp", bufs=3))
    smp = ctx.enter_context(tc.tile_pool(name="smp", bufs=2))
    stp = ctx.enter_context(tc.tile_pool(name="stp", bufs=2, space="PSUM"))
    otp = ctx.enter_context(tc.tile_pool(name="otp", bufs=1, space="PSUM"))
    qkp = ctx.enter_context(tc.tile_pool(name="qkp", bufs=2, space="PSUM"))

    # ---- persistent SBUF tensors ----
    x_sb = [pers.tile([128, L], BF16, tag=f"x{c}", name=f"x{c}") for c in range(NT)]
    wq_sb = [
        pers.tile([128, 3 * HID], BF16, tag=f"wq{c}", name=f"wq{c}") for c in range(NT)
    ]
    wo_sb = [pers.tile([128, C], BF16, tag=f"wo{c}", name=f"wo{c}") for c in range(NT)]
    bias_sb = [
        pers.tile([128, 1], F32, tag=f"bias{c}", name=f"bias{c}") for c in range(NT)
    ]
    q2 = [pers.tile([128, L], BF16, tag=f"q2_{h}", name=f"q2_{h}") for h in range(H)]
    k2 = [pers.tile([128, L], BF16, tag=f"k2_{h}", name=f"k2_{h}") for h in range(H)]
    vt1 = [
        pers.tile([128, H * 65], BF16, tag=f"vt{j}", name=f"vt{j}") for j in range(NJ)
    ]
    o2 = [pers.tile([128, L], BF16, tag=f"o2_{c}", name=f"o2_{c}") for c in range(NT)]

    # ---- input DMAs (split into halves so more DMA queues run in parallel;
    # wo/bias are only needed at the projection and are loaded later) ----
    for c in range(NT):
        r = slice(128 * c, 128 * (c + 1))
        nc.sync.dma_start(x_sb[c][:, 0:1024], x_d[r, 0:1024])
        nc.sync.dma_start(x_sb[c][:, 1024:2048], x_d[r, 1024:2048])
        nc.sync.dma_start(wq_sb[c][:, 0:1024], wqkvT_d[r, 0:1024])
        nc.sync.dma_start(wq_sb[c][:, 1024:1536], wqkvT_d[r, 1024:1536])

    def emit_qk_pair(t):
        """Project q and k rows 128t..128t+128 (heads 2t, 2t+1) and write the
        partition-duplicated q2/k2 tiles via SBUF->SBUF DMA."""
        for kind in range(2):  # 0 = q, 1 = k
            stage = stg.tile([128, L], BF16, tag="stage", name=f"stage_{kind}_{t}")
            ocol = kind * HID + 128 * t
            for n in range(NL):
                ps = qkp.tile([128, 512], F32, tag="qkp", name=f"qk_ps_{kind}_{t}_{n}")
                for c in range(NT):
                    nc.tensor.matmul(
                        ps[:, :],
                        lhsT=wq_sb[c][:, ocol : ocol + 128],
                        rhs=x_sb[c][:, 512 * n : 512 * (n + 1)],
                        start=(c == 0),
                        stop=(c == NT - 1),
                    )
                nc.vector.tensor_copy(stage[:, 512 * n : 512 * (n + 1)], ps[:, :])
            dsts = (q2, k2)[kind]
            d0, d1 = dsts[2 * t], dsts[2 * t + 1]
            nc.sync.dma_start(d0[0:64, :], stage[0:64, :])
            nc.sync.dma_start(d0[64:128, :], stage[0:64, :])
            nc.sync.dma_start(d1[0:64, :], stage[64:128, :])
            nc.sync.dma_start(d1[64:128, :], stage[64:128, :])

    def emit_vt(jt):
        """V^T tile for key-block jt: [128 keys, 8 heads x (64 dims + ones)]."""
        ps = qkp.tile([128, 512], F32, tag="qkp", name=f"vt_ps_{jt}")
        for c in range(NT):
            nc.tensor.matmul(
                ps[:, :],
                lhsT=x_sb[c][:, 128 * jt : 128 * (jt + 1)],
                rhs=wq_sb[c][:, 2 * HID : 3 * HID],
                start=(c == 0),
                stop=(c == NT - 1),
            )
        vv = vt1[jt].rearrange("p (h e) -> p h e", e=65)
        nc.vector.tensor_copy(vv[:, :, 0:64], ps.rearrange("p (h d) -> p h d", d=64))
        nc.vector.memset(vv[:, :, 64:65], 1.0)

    def emit_head(h, ih, interleave):
        """Attention for head h, i-half ih. `interleave` is a list of closures
        emitting independent PE work (qk-proj of later pairs / proj groups) to
        fill PE slack inside the exp-bound j-loop."""
        t = h // 2
        q2h, k2h = q2[h], k2[h]
        slot = 0
        ib = 1024 * ih
        ot = otp.tile([65, 1024], F32, tag="ot", name=f"ot_{h}_{ih}")
        for jt in range(NJ):
            st = stp.tile([128, 1024], F32, tag="st", name=f"st_{h}_{ih}_{jt}")
            # packed S^T pair: rows 0-63 compute i-chunk 0, rows 64-127
            # compute i-chunk 1 (concurrent in the PE array)
            nc.tensor.matmul(
                st[:, 0:512],
                lhsT=k2h[0:64, 128 * jt : 128 * (jt + 1)],
                rhs=q2h[0:64, ib : ib + 512],
                start=True,
                stop=True,
            )
            nc.tensor.matmul(
                st[:, 512:1024],
                lhsT=k2h[64:128, 128 * jt : 128 * (jt + 1)],
                rhs=q2h[64:128, ib + 512 : ib + 1024],
                start=True,
                stop=True,
            )
            pt = ptp.tile([128, 1024], BF16, tag="pt", name=f"pt_{h}_{ih}_{jt}")
            nc.scalar.activation(pt[:, :], st[:, :], AF.Exp, scale=SCALE)
            vt = vt1[jt]
            nc.tensor.matmul(
                ot[:, 0:512],
                lhsT=vt[:, 65 * h : 65 * h + 65],
                rhs=pt[:, 0:512],
                start=(jt == 0),
                stop=(jt == NJ - 1),
            )
            nc.tensor.matmul(
                ot[:, 512:1024],
                lhsT=vt[:, 65 * h : 65 * h + 65],
                rhs=pt[:, 512:1024],
                start=(jt == 0),
                stop=(jt == NJ - 1),
            )
            # fill PE slack with independent work (one closure per j-tile)
            if slot < len(interleave):
                interleave[slot]()
                slot += 1
        # Evacuate the accumulator in ONE copy so the psum tile frees fast
        # (otherwise the next head's first PV stalls ~5us and the PE goes
        # HAM-cold); normalize afterwards from SBUF.
        # NOTE: reciprocal_approx_fast mis-reads PSUM at partition offset 64
        # on silicon (reads partition 0), so SBUF staging is also required.
        o2u = smp.tile([65, 1024], F32, tag="o2u", name=f"o2u_{h}_{ih}")
        nc.vector.tensor_copy(o2u[:, :], ot[:, :])
        den = smp.tile([1, 1024], F32, tag="den", name=f"den_{h}_{ih}")
        nc.vector.tensor_copy(den[:, :], o2u[64:65, :])
        rec = smp.tile([1, 1024], F32, tag="rec", name=f"rec_{h}_{ih}")
        nc.vector.reciprocal_approx_fast(rec[:, :], den[:, :])
        rb = smp.tile([64, 1024], F32, tag="rb", name=f"rb_{h}_{ih}")
        nc.gpsimd.partition_broadcast(rb[:, :], rec[:, :])
        dst = o2[t][(h % 2) * 64 : (h % 2) * 64 + 64, ib : ib + 1024]
        nc.vector.tensor_mul(dst, o2u[0:64, :], rb[:, :])
        del interleave[:slot]

    def emit_proj_group(o, n):
        ps = qkp.tile([128, 512], F32, tag="qkp", name=f"y_ps_{o}_{n}")
        for c in range(NT):
            nc.tensor.matmul(
                ps[:, :],
                lhsT=wo_sb[c][:, 128 * o : 128 * (o + 1)],
                rhs=o2[c][:, 512 * n : 512 * (n + 1)],
                start=(c == 0),
                stop=(c == NT - 1),
            )
        yt = ytp.tile([128, 512], F32, tag="yt", name=f"yt_{o}_{n}")
        nc.vector.tensor_scalar_add(yt[:, :], ps[:, :], bias_sb[o][:, 0:1])
        nc.sync.dma_start(
            out_d[128 * o : 128 * (o + 1), 512 * n : 512 * (n + 1)], yt[:, :]
        )

    # ---- emission schedule ----
    emit_qk_pair(0)  # heads 0,1 projected up front
    for jt in range(4):
        emit_vt(jt)  # first few V^T tiles; the rest interleave into head 0

    # load wo/bias now (off the critical startup path)
    for c in range(NT):
        r = slice(128 * c, 128 * (c + 1))
        nc.sync.dma_start(wo_sb[c][:, :], woutT_d[r, :])
        nc.sync.dma_start(bias_sb[c][:, :], bias_d[r, :])

    # ih-outer: pass 0 computes o2[:, 0:1024] for all heads, so the first
    # half of the output projection overlaps pass 1. qk pairs for later
    # heads are interleaved into earlier heads' j-loops so the PE fills its
    # slack while ScalarE exp is the bottleneck.
    inter_by_head = {
        0: [lambda jt=jt: emit_vt(jt) for jt in range(4, NJ)],
        1: [lambda: emit_qk_pair(1)],
        2: [lambda: emit_qk_pair(2)],
        3: [lambda: emit_qk_pair(3)],
    }
    for h in range(H):
        emit_head(h, 0, inter_by_head.get(h, []))
    proj_half0 = [
        lambda o=o, n=n: emit_proj_group(o, n) for o in range(NT) for n in (0, 1)
    ]
    for h in range(H):
        inter = [proj_half0.pop(0)] if proj_half0 else []
        emit_head(h, 1, inter)
    for o in range(NT):
        for n in (2, 3):
            emit_proj_group(o, n)
    ctx.close()


_COMPILED = None


def _get_compiled():
    global _COMPILED
    if _COMPILED is None:
        nc = bacc.Bacc(
            "TRN2", target_bir_lowering=False, debug=False, num_devices=NCORES
        )
        x_d = nc.dram_tensor("x", [C, L], BF16, kind="ExternalInput").ap()
        wqkvT_d = nc.dram_tensor("wqkvT", [C, 3 * HID], BF16, kind="ExternalInput").ap()
        woutT_d = nc.dram_tensor("woutT", [HID, C], BF16, kind="ExternalInput").ap()
        bias_d = nc.dram_tensor("bias", [C, 1], F32, kind="ExternalInput").ap()
        out_d = nc.dram_tensor("out", [C, L], F32, kind="ExternalOutput").ap()
        with tile.TileContext(nc) as tc:
            build_kernel(tc, out_d, x_d, wqkvT_d, woutT_d, bias_d)
        nc.compile()
        _COMPILED = nc
    return _COMPILED


def make_in_maps(x, w_qkv, w_out, b_out):
    xb = np.asarray(x, dtype=np.float32).astype(ml_dtypes.bfloat16)
    wqkvT = np.ascontiguousarray(
        np.asarray(w_qkv, dtype=np.float32).T.astype(ml_dtypes.bfloat16)
    )
    woutT = np.ascontiguousarray(
        np.asarray(w_out, dtype=np.float32).T.astype(ml_dtypes.bfloat16)
    )
    bias = np.ascontiguousarray(
        np.asarray(b_out, dtype=np.float32).reshape(C, 1)
    )
    return [
        {
            "x": np.ascontiguousarray(xb[b]),
            "wqkvT": wqkvT,
            "woutT": woutT,
            "bias": bias,
        }
        for b in range(B)
    ]


LAST_RESULTS = None


def _install_ntff_hook():
    """Provide antenv.axon_hooks (absent from this image) so trace=True works."""
    import types

    try:
        from antenv.axon_hooks import get_axon_ntff_profile_hook  # noqa: F401

        return
    except ImportError:
        pass
    sys.path.insert(0, "/root/.axon_site")
    from trn_agent_boot.trn_boot import _ntff_profile_via_ctypes

    hook = _ntff_profile_via_ctypes("/opt/axon/libaxon_pjrt.so")
    import antenv

    mod = types.ModuleType("antenv.axon_hooks")
    mod._hook = hook
    mod.get_axon_ntff_profile_hook = lambda: mod._hook
    mod.set_axon_ntff_profile_hook = lambda h: setattr(mod, "_hook", h)
    sys.modules["antenv.axon_hooks"] = mod
    antenv.axon_hooks = mod
    # artifact upload has no egress in this container - make it a no-op
    bass_utils.upload_artifacts = lambda tmpdir: tmpdir


def kernel(x, w_qkv, w_out, b_out):
    global LAST_RESULTS
    nc = _get_compiled()
    in_maps = make_in_maps(x, w_qkv, w_out, b_out)
    trace = bool(int(os.environ.get("KERNEL_TRACE", "0")))
    if trace:
        _install_ntff_hook()
    res = bass_utils.run_bass_kernel_spmd(
        nc, in_maps, core_ids=list(range(NCORES)), trace=trace
    )
    LAST_RESULTS = res
    out = np.stack([np.asarray(res.results[b]["out"]) for b in range(B)])
    return out.astype(np.float32)
